# revision 6
# baseline (speedup 1.0000x reference)
"""Trainium2 Bass kernel for fused CLIP text-image per-class heads.

Problem: B=256 images [256,768]; K=512 heads (312 adj + 200 noun). Per head k:
  lin[b,k] = img[b]·lin_W[k,:768] + txt[k]·lin_W[k,768:] + lin_b[k]
  h[b,k,:] = img[b] @ W1[k,:768,:] + txt[k] @ W1[k,768:,:] + b1[k]   # [B,312]
  h = BN_train(relu(h)) * gamma + beta   (batch stats over B)
  cls[b,k] = h[b,k,:]·W2[k] + b2[k]

Sharding: heads split 64-per-core across 8 cores (expert parallel). BN is over
the batch dim, which stays whole on every core -> no cross-device collectives;
the host concatenates per-core [64,B] outputs and transposes.

Per-core layout: H (=312, 3 chunks of 128/128/56) on partitions, batch on the
free axis, so BN stats are free-dim reductions. The main matmul streams the
whole W1 slab (102 MB/core) as stationary operands: for each head,
out[Hc,256] += W1[dchunk,Hc].T @ rhs[dchunk,256], where rhs is imgT for the
768 image rows and txt[k] broadcast across the batch for the 512 text rows
(the concat is never materialized). Matmul operands are float32r (PE full
rate, 1 cycle/row for moving dim >=256, ~2^-12 mantissa): HBM-resident ones
are cast-rounded for free by gpsimd DMA, on-chip ones are rounded by the
compute op that produces them. BatchNorm is applied as an affine (scale,bias)
fused into one scalar-engine pass, then cls is a [Hc,1].T @ [Hc,256] matmul
accumulated over chunks. All host->device layouts are partition-major so
every DMA is a [128, contiguous] transfer.
"""

import numpy as np

IMG_D, TXT_D, HID = 768, 512, 312
B = 256
A_HEADS, N_HEADS = 312, 200
K_TOT = A_HEADS + N_HEADS  # 512
NCORES = 8
PER = K_TOT // NCORES  # 64 heads per core
G = 8  # heads per stats group
BN_EPS = 1e-5
HCH = [(0, 128), (128, 128), (256, 56)]  # HID chunk (offset, size)
ND = (IMG_D + TXT_D) // 128  # 10 contraction chunks

_CACHE = {}


def _split_multiwait(nc, limit=1):
    """This walrus build rejects >1 sync wait on one instruction
    (CoreV3 setupSyncWait 'Too many sync wait commands'). Split excess
    waits onto Drain instructions inserted just before the offender."""
    from concourse import mybir

    n_new = 0
    for fn in nc.m.functions:
        for blk in fn.blocks:
            out = []
            changed = False
            for ins in list(blk.instructions):
                si = ins.sync_info
                if si is not None and len(si.on_wait) > limit:
                    waits = list(si.on_wait)
                    extra, keep = waits[:-limit], waits[-limit:]
                    for i in range(0, len(extra), limit):
                        n_new += 1
                        out.append(
                            mybir.InstDrain(
                                name=f"{ins.name}-wsplit{n_new}",
                                engine=ins.engine,
                                ins=[],
                                outs=[],
                                sync_info=mybir.SyncInfo(
                                    on_wait=extra[i : i + limit], on_update=[]
                                ),
                            )
                        )
                    ins.sync_info = mybir.SyncInfo(
                        on_wait=keep, on_update=list(si.on_update)
                    )
                    changed = True
                out.append(ins)
            if changed:
                blk.instructions = out
    return n_new


def build_nc():
    import concourse.bass as bass
    import concourse.tile as tile
    from contextlib import ExitStack
    from concourse import mybir

    f32 = mybir.dt.float32
    f32r = mybir.dt.float32r
    AF = mybir.ActivationFunctionType

    nc = bass.Bass(trn_type="TRN2")

    # all DRAM layouts are already partition-major ([.., 128, ..]-contiguous)
    img_l = nc.dram_tensor("img_l", [128, 6, B], f32, kind="ExternalInput")
    w1 = nc.dram_tensor("w1", [PER, 128, ND, HID], f32, kind="ExternalInput")
    txtT = nc.dram_tensor("txtT", [128, 4, PER], f32, kind="ExternalInput")
    txt = nc.dram_tensor("txt", [PER, TXT_D], f32, kind="ExternalInput")
    lwiT = nc.dram_tensor("lwiT", [128, 6, PER], f32, kind="ExternalInput")
    lwt = nc.dram_tensor("lwt", [PER, TXT_D], f32, kind="ExternalInput")
    linb = nc.dram_tensor("linb", [PER, 1], f32, kind="ExternalInput")
    b1T = nc.dram_tensor("b1T", [128, 3, PER], f32, kind="ExternalInput")
    gamT = nc.dram_tensor("gamT", [128, 3, PER], f32, kind="ExternalInput")
    betT = nc.dram_tensor("betT", [128, 3, PER], f32, kind="ExternalInput")
    w2g = nc.dram_tensor("w2g", [128, 3, PER, G], f32, kind="ExternalInput")
    b2g = nc.dram_tensor("b2g", [G, PER // G], f32, kind="ExternalInput")

    lin_o = nc.dram_tensor("lin_o", [PER, B], f32, kind="ExternalOutput")
    cls_o = nc.dram_tensor("cls_o", [PER, B], f32, kind="ExternalOutput")

    with tile.TileContext(nc) as tc, ExitStack() as ctx:
        const = ctx.enter_context(tc.tile_pool(name="const", bufs=1))
        wpool = ctx.enter_context(tc.tile_pool(name="w1p", bufs=3))
        tbpool = ctx.enter_context(tc.tile_pool(name="tbp", bufs=3))
        rgpool = ctx.enter_context(tc.tile_pool(name="rgp", bufs=2))
        stpool = ctx.enter_context(tc.tile_pool(name="stp", bufs=2))
        hpsum = ctx.enter_context(tc.tile_pool(name="hps", bufs=2, space="PSUM"))
        cpsum = ctx.enter_context(tc.tile_pool(name="cps", bufs=2, space="PSUM"))

        # ---- constants / one-time loads (gpsimd DMAs cast-round to f32r) ----
        img_sb = const.tile([128, 6, B], f32r)
        nc.gpsimd.dma_start(out=img_sb, in_=img_l[:, :, :])
        txtT_sb = const.tile([128, 4, PER], f32)
        nc.sync.dma_start(out=txtT_sb, in_=txtT[:, :, :])
        txt_sb = const.tile([PER, TXT_D], f32)
        nc.sync.dma_start(out=txt_sb, in_=txt[:, :])
        lwiT_sb = const.tile([128, 6, PER], f32r)
        nc.gpsimd.dma_start(out=lwiT_sb, in_=lwiT[:, :, :])
        lwt_sb = const.tile([PER, TXT_D], f32)
        nc.sync.dma_start(out=lwt_sb, in_=lwt[:, :])
        linb_sb = const.tile([PER, 1], f32)
        nc.sync.dma_start(out=linb_sb, in_=linb[:, :])
        b2g_sb = const.tile([G, PER // G], f32)
        nc.sync.dma_start(out=b2g_sb, in_=b2g[:, :])
        b1T_sb = const.tile([128, 3, PER], f32)
        nc.sync.dma_start(out=b1T_sb, in_=b1T[:, :, :])
        gamT_sb = const.tile([128, 3, PER], f32)
        nc.sync.dma_start(out=gamT_sb, in_=gamT[:, :, :])
        betT_sb = const.tile([128, 3, PER], f32)
        nc.sync.dma_start(out=betT_sb, in_=betT[:, :, :])
        w2g_sb = const.tile([128, 3, PER, G], f32r)
        nc.gpsimd.dma_start(out=w2g_sb, in_=w2g[:, :, :, :])

        ones = const.tile([128, B], f32)
        nc.vector.memset(ones, 1.0)
        eps_sb = const.tile([128, 1], f32)
        nc.vector.memset(eps_sb, BN_EPS)

        # ---- lin head: [64,B] = linWimg @ imgT + (txt*linWtxt).sum + lin_b ----
        lt = const.tile([PER, TXT_D], f32)
        nc.vector.tensor_mul(out=lt, in0=txt_sb, in1=lwt_sb)
        ls = const.tile([PER, 1], f32)
        nc.vector.reduce_sum(out=ls, in_=lt, axis=mybir.AxisListType.X)
        ls2 = const.tile([PER, 1], f32)
        nc.vector.tensor_add(out=ls2, in0=ls, in1=linb_sb)
        lin_ps = cpsum.tile([PER, B], f32, tag="cps")
        for dc in range(6):
            nc.tensor.matmul(
                out=lin_ps,
                lhsT=lwiT_sb[:, dc, :],
                rhs=img_sb[:, dc, :],
                start=(dc == 0),
                stop=(dc == 5),
            )
        lin_sb = const.tile([PER, B], f32)
        nc.vector.tensor_scalar_add(out=lin_sb, in0=lin_ps, scalar1=ls2)
        nc.sync.dma_start(out=lin_o[:, :], in_=lin_sb)

        # ---- main loop over head groups ----
        for g0 in range(0, PER, G):
            rg = [
                rgpool.tile([128, G, B], f32r, name=f"rg{c}", tag=f"rg{c}")
                for c in range(3)
            ]
            stats = stpool.tile([128, 3, G, 6], f32, tag="stats")
            mv = stpool.tile([128, 3, G, 2], f32, tag="mv")
            sd = stpool.tile([128, 3, G], f32, tag="sd")
            inv = stpool.tile([128, 3, G], f32, tag="inv")
            sg = stpool.tile([128, 3, G], f32, tag="sg")
            tg = stpool.tile([128, 3, G], f32, tag="tg")

            for gi in range(G):
                k = g0 + gi
                w1sb = wpool.tile([128, ND, HID], f32r, tag="w1sb")
                nc.gpsimd.dma_start(out=w1sb, in_=w1[k, :, :, :])
                tb = tbpool.tile([128, 4, B], f32r, tag="tb")
                for j in range(4):
                    nc.vector.tensor_scalar_mul(
                        out=tb[:, j, :], in0=ones, scalar1=txtT_sb[:, j, k : k + 1]
                    )
                for ci, (h0, hn) in enumerate(HCH):
                    hp = hpsum.tile([128, B], f32, tag=f"hps{ci}")
                    for dc in range(ND):
                        rhs = img_sb[:, dc, :] if dc < 6 else tb[:, dc - 6, :]
                        nc.tensor.matmul(
                            out=hp[:hn, :],
                            lhsT=w1sb[:, dc, h0 : h0 + hn],
                            rhs=rhs,
                            start=(dc == 0),
                            stop=(dc == ND - 1),
                        )
                    # relu(h + b1), PSUM -> SBUF group tile (rounds to f32r)
                    nc.scalar.activation(
                        out=rg[ci][:hn, gi, :],
                        in_=hp[:hn, :],
                        func=AF.Relu,
                        bias=b1T_sb[:hn, ci, k : k + 1],
                        scale=1.0,
                    )

            # ---- group batch-norm stats (over batch = free dim) ----
            for ci, (h0, hn) in enumerate(HCH):
                for gi in range(G):  # walrus: bn_stats out must be 6 el/partition
                    nc.vector.bn_stats(
                        out=stats[:hn, ci, gi, :], in_=rg[ci][:hn, gi, :]
                    )
                for gi in range(G):
                    nc.vector.bn_aggr(
                        out=mv[:hn, ci, gi, :], in_=stats[:hn, ci, gi : gi + 1, :]
                    )
            # scale = gamma/sqrt(var+eps); bias = beta - mean*scale
            nc.scalar.activation(
                out=sd, in_=mv[:, :, :, 1], func=AF.Sqrt, bias=eps_sb, scale=1.0
            )
            nc.vector.reciprocal(out=inv, in_=sd)
            nc.vector.tensor_mul(out=sg, in0=inv, in1=gamT_sb[:, :, g0 : g0 + G])
            nc.vector.tensor_mul(out=tg, in0=mv[:, :, :, 0], in1=sg)
            nc.vector.tensor_sub(out=tg, in0=betT_sb[:, :, g0 : g0 + G], in1=tg)

            # ---- normalize in place (rounds to f32r) + cls matmul ----
            # all G heads accumulate into one [G,B] psum via one-hot-column
            # W2 stationaries (w2g[:, c, k, :] is zero except column k%G)
            cls_gps = cpsum.tile([G, B], f32, tag="cps")
            for gi in range(G):
                k = g0 + gi
                for ci, (h0, hn) in enumerate(HCH):
                    nc.scalar.activation(
                        out=rg[ci][:hn, gi, :],
                        in_=rg[ci][:hn, gi, :],
                        func=AF.Identity,
                        scale=sg[:hn, ci, gi : gi + 1],
                        bias=tg[:hn, ci, gi : gi + 1],
                    )
                    nc.tensor.matmul(
                        out=cls_gps,
                        lhsT=w2g_sb[:hn, ci, k, :],
                        rhs=rg[ci][:hn, gi, :],
                        start=(gi == 0 and ci == 0),
                        stop=(gi == G - 1 and ci == 2),
                    )
            cls_gsb = stpool.tile([G, B], f32, tag="clsg")
            nc.vector.tensor_scalar_add(
                out=cls_gsb, in0=cls_gps, scalar1=b2g_sb[:, g0 // G : g0 // G + 1]
            )
            nc.sync.dma_start(out=cls_o[g0 : g0 + G, :], in_=cls_gsb)

    _split_multiwait(nc)
    return nc


def _pmajor(x, nchunk):
    """[nchunk*128, F...] -> [128, nchunk, F...] partition-major contiguous."""
    x = np.asarray(x, np.float32)
    return np.ascontiguousarray(
        x.reshape(nchunk, 128, *x.shape[1:]).swapaxes(0, 1)
    )


def _shard_inputs(
    image_out,
    adj_text,
    noun_text,
    adj_lin_W,
    adj_lin_b,
    adj_W1,
    adj_b1,
    adj_gamma,
    adj_beta,
    adj_W2,
    adj_b2,
    noun_lin_W,
    noun_lin_b,
    noun_W1,
    noun_b1,
    noun_gamma,
    noun_beta,
    noun_W2,
    noun_b2,
):
    f = np.float32
    cat = lambda a, b: np.concatenate([np.asarray(a, f), np.asarray(b, f)], axis=0)
    W1 = cat(adj_W1, noun_W1)  # [512, 1280, 312]
    txt = cat(adj_text, noun_text)  # [512, 512]
    linW = cat(adj_lin_W, noun_lin_W)  # [512, 1280]
    linb = cat(adj_lin_b, noun_lin_b)  # [512]
    b1 = cat(adj_b1, noun_b1)  # [512, 312]
    gam = cat(adj_gamma, noun_gamma)
    bet = cat(adj_beta, noun_beta)
    W2 = cat(adj_W2, noun_W2)  # [512, 312]
    b2 = cat(adj_b2, noun_b2)  # [512]

    img_l = _pmajor(np.asarray(image_out, f).T, 6)  # [128, 6, 256]

    def padT(x):  # [64, HID] -> [128, 3, 64]
        p = np.zeros((384, PER), f)
        p[:HID] = x.T
        return _pmajor(p, 3)

    maps = []
    for c in range(NCORES):
        hs = slice(c * PER, (c + 1) * PER)
        # W1 slab -> [64, 128, 10, 312]: partition-major within each head
        w1c = np.ascontiguousarray(
            W1[hs].reshape(PER, ND, 128, HID).swapaxes(1, 2)
        )
        w2t3 = padT(W2[hs])  # [128, 3, 64]
        w2gc = np.zeros((128, 3, PER, G), f)
        idx = np.arange(PER)
        w2gc[:, :, idx, idx % G] = w2t3
        maps.append(
            {
                "img_l": img_l,
                "w1": w1c,
                "txtT": _pmajor(np.ascontiguousarray(txt[hs].T), 4),
                "txt": np.ascontiguousarray(txt[hs]),
                "lwiT": _pmajor(np.ascontiguousarray(linW[hs, :IMG_D].T), 6),
                "lwt": np.ascontiguousarray(linW[hs, IMG_D:]),
                "linb": np.ascontiguousarray(linb[hs][:, None]),
                "b1T": padT(b1[hs]),
                "gamT": padT(gam[hs]),
                "betT": padT(bet[hs]),
                "w2g": w2gc,
                "b2g": np.ascontiguousarray(b2[hs].reshape(PER // G, G).T),
            }
        )
    return maps


def _assemble(results):
    lin = np.concatenate([r["lin_o"] for r in results], axis=0)  # [512, 256]
    cls = np.concatenate([r["cls_o"] for r in results], axis=0)
    linT = np.ascontiguousarray(lin.T)  # [256, 512]
    clsT = np.ascontiguousarray(cls.T)
    return (
        linT[:, :A_HEADS],
        linT[:, A_HEADS:],
        clsT[:, :A_HEADS],
        clsT[:, A_HEADS:],
    )


def kernel(**inputs):
    from concourse.bass_utils import run_bass_kernel_spmd

    if "nc" not in _CACHE:
        _CACHE["nc"] = build_nc()
    in_maps = _shard_inputs(**inputs)
    res = run_bass_kernel_spmd(_CACHE["nc"], in_maps, core_ids=list(range(NCORES)))
    return _assemble(res.results)
